# revision 59
# baseline (speedup 1.0000x reference)
"""Causal self-attention on 8 Trainium2 NeuronCores (Bass/Tile).

Problem: x[4,2048,1024] @ W_attn[1024,3072] + b_attn -> qkv; 16-head causal
attention; y @ W_proj[1024,1024] + b_proj.

Sharding: 2D over (batch, head-group), zero inter-core communication.
Core c = (b = c//2, g = c%2); each core computes q/k/v for its 8 heads over
its batch, flash-style causal attention (no max subtraction -- logits are
small), then a partial output projection with its 512-row slice of W_proj.
The host adds the two partials per batch plus b_proj and the v-bias
projection bv @ W_proj (softmax weights sum to 1, so the v-bias passes
through attention unchanged and never needs to enter the kernel).

Kernel structure (build_nc3), tuned against TimelineSim + HW reps-slope:
  - x arrives host-pre-transposed ([feat%128, feat//128, tok]): the XBAR
    transpose DMAs of the previous version cost ~4.5-7us of SP sequencer
    time EACH (16 of them); plain strided loads don't.
  - denominator broadcast in the matmul: v_aug[:, j, h] = [v | 64 ones
    columns], so the AV matmul replicates the softmax denominator across
    PSUM partitions 64..127 and normalization is reciprocal[64,TCH] +
    mul per head -- no 1-partition reciprocal, no gpsimd
    partition_broadcast (DVE may read only ONE PSUM input per op).
  - PSUM layout: ps pool = 2 x 2-bank score bufs (paired QK needs the two
    heads' regions in separate banks); flex pool = 4 x 1-bank bufs shared
    by projection groups, output-projection units and the per-head AV
    accumulators py_a/py_b (freed right after the 2-op normalize, so pair
    boundaries don't stall).  8 banks exactly.
  - head-paired QK: heads (2i, 2i+1) at partitions 0-63/64-127; K=64
    matmuls emitted adjacently run concurrently via PE row tiling (2x on
    HW; not modeled by TimelineSim).  AV cannot pair: 2x(64 v + ones) >
    128 PE columns, and every alternative denominator computation costs
    >= the pairing gain (PSUM output is capped at one 2KB bank).
  - filler-pumped emission: next-chunk projection groups and deferred
    output-projection units are generators yielding one matmul per step,
    pumped between QK(j+lead) and AV(j) (in-order queues: what's emitted
    between them is what the PE executes while ACT runs exp).  The QK
    emission leads AV by 3 key blocks (est_bufs=4 holds lead+1 tiles),
    which removed most mid-kernel PE stalls (-5us sim vs lead=1).  All
    output-projection units are deferred into the LAST chunk's attention
    window -- its 64 ACT-bound j-steps have no other PE filler -- paced
    at ~the ACT-PE deficit (1 matmul/step).
  - DMA: weights on the ACT HWDGE queue, x and output stores on SP (two
    parallel descriptor streams; ~2 DMAs in flight each); wv sliced per
    2 contraction blocks with the chunk-0 v-projection running d-major
    over 4 parallel PSUM groups so each slice is consumed on arrival;
    tail stores/copies alternate ACT/DVE+SP queues.
  - causal trimming on diagonal blocks (QK/exp/mask/AV over the valid
    query columns only); masks multiply post-exp on DVE (Pool
    affine_select was tried: 0.42-0.6 efficiency puts it on the
    exp->AV critical chain, +21us sim).

TimelineSim: 246us (PE 92% busy; ACT 149us, DVE 117us) vs 270us for
build_nc2.  HW reps-slope (reps 2 vs 16, interleaved, median):
~270-330us/rep vs ~420-440us for build_nc2, i.e. ~30-35% faster;
rel err 3.6e-3 vs fp32 reference (tolerance 2e-2).
"""

import numpy as np

import concourse.mybir as mybir
import concourse.tile as tile
from concourse import bacc
from concourse.bass_utils import run_bass_kernel_spmd

F32 = mybir.dt.float32
F32R = mybir.dt.float32r
BF16 = mybir.dt.bfloat16

B, T, D, H = 4, 2048, 1024, 16
HD = D // H               # 64
N_GROUPS = 2
FQ = D // N_GROUPS        # 512 features (8 heads) per core
N_CORES = B * N_GROUPS

# set by test harness to collect an NTFF trace / HW exec time
TRACE = False
LAST_RESULTS = None


def build_nc3(reps=1, x_dt=BF16, qk_dt=BF16, av_dt=BF16, pj_dt=BF16,
              est_bufs=4, out_bufs=8, trim=True, out_dt=BF16, TCH=512,
              fill_per_j=2, flex_bufs=4, pair_fill=2, exp_half=False,
              qk_nopair=False, qk_lead=3):
    """Chunk-pipelined causal attention, v2 schedule.

    Differences from build_nc2:
      - one 4-buf 1-bank "flex" PSUM pool serves projection groups, output
        projection and the per-head AV accumulators (py_a/py_b), replacing
        the pj/py pools; ps keeps 2x2-bank score bufs.  AV accumulators
        free right after a single divide, so pair boundaries don't stall.
      - v_aug carries 64 replicated ones-columns: the AV matmul broadcasts
        the softmax denominator across PSUM partitions 64..127, so
        normalization is one tensor_tensor(divide) per head -- no 1-partition
        reciprocal, no gpsimd partition_broadcast, shorter tail chain.
      - next-chunk projections and prev-chunk output-projection units are
        emitted as matmul-granular fillers woven into the attention j-loop
        (pump()), keeping PE busy while ACT works through exp.
      - weight DMAs are sliced per contraction block so the first
        projection matmuls start as soon as their slice lands.
    """
    P = 128
    NTC = T // TCH            # token chunks
    TBC = TCH // P            # token blocks per chunk
    DCH = D // P              # contraction blocks
    NFB = FQ // P             # feature blocks of qT/kT
    HLOC = FQ // HD           # heads on this core
    NPAIR = HLOC // 2
    NTB = T // P              # key blocks total
    NLC = FQ // P
    DOUT_CH = 512
    NDOUT = D // DOUT_CH
    scale = 1.0 / float(np.sqrt(HD))

    nc = bacc.Bacc()
    # x arrives host-pre-transposed as [feat%128, feat//128, tok]: XBAR
    # transpose DMAs cost ~4.5-7us of SP sequencer time each, plain strided
    # loads don't
    xbT = nc.dram_tensor("xbT", [P, DCH, T], x_dt, kind="ExternalInput")
    wq = nc.dram_tensor("wq", [D, FQ], x_dt, kind="ExternalInput")
    wk = nc.dram_tensor("wk", [D, FQ], x_dt, kind="ExternalInput")
    wv = nc.dram_tensor("wv", [D, FQ], x_dt, kind="ExternalInput")
    bq = nc.dram_tensor("bq", [FQ], F32, kind="ExternalInput")
    bk = nc.dram_tensor("bk", [FQ], F32, kind="ExternalInput")
    wp = nc.dram_tensor("wp", [FQ, D], pj_dt, kind="ExternalInput")
    out = nc.dram_tensor("out", [T, D], out_dt, kind="ExternalOutput")

    with tile.TileContext(nc) as tc:
        with (
            tc.tile_pool(name="const", bufs=1) as const,
            tc.tile_pool(name="big", bufs=1) as big,
            tc.tile_pool(name="xtp", bufs=2) as xtp,
            tc.tile_pool(name="est", bufs=est_bufs) as est,
            tc.tile_pool(name="small", bufs=2) as small,
            tc.tile_pool(name="outp", bufs=out_bufs) as outp,
            tc.tile_pool(name="ps", bufs=2, space="PSUM") as ps,
            tc.tile_pool(name="flex", bufs=flex_bufs, space="PSUM") as flex,
        ):
            # diagonal-block masks: mask_r[p, f] = 1 if f >= p + 128*r else 0
            masks = []
            for r in range(TBC):
                m = const.tile([P, TCH], av_dt, tag=f"mask{r}")
                nc.gpsimd.memset(m, 1.0)
                nc.gpsimd.affine_select(
                    out=m, in_=m,
                    compare_op=mybir.AluOpType.is_ge,
                    fill=0.0,
                    base=-P * r,
                    pattern=[[1, TCH]],
                    channel_multiplier=-1,
                )
                masks.append(m)
            # weight DMAs issue from the ACT queue (idle until the first
            # exp), x loads from SP: two HWDGE streams in parallel
            xpre = xtp.tile([P, DCH, TCH], x_dt, tag="xT")
            nc.sync.dma_start(out=xpre[:, :, 0:P], in_=xbT[:, :, 0:P])
            wv_sb = big.tile([P, DCH, FQ], x_dt, tag="wv_sb")
            wv_r = wv.rearrange("(dc p) f -> p dc f", p=P)
            # fine slices at the head of the queue so the first projection
            # groups unblock as early as possible; ~2 DMAs in flight per
            # queue
            for (d0, d1) in ((0, 2), (2, 4), (4, 6), (6, 8)):
                nc.scalar.dma_start(out=wv_sb[:, d0:d1, :],
                                    in_=wv_r[:, d0:d1, :])
            nc.sync.dma_start(out=xpre[:, :, P:TCH], in_=xbT[:, :, P:TCH])
            wq_sb = big.tile([P, DCH, FQ], x_dt, tag="wq_sb")
            wk_sb = big.tile([P, DCH, FQ], x_dt, tag="wk_sb")
            bq_sb = const.tile([P, NFB], F32, tag="bq")
            bk_sb = const.tile([P, NFB], F32, tag="bk")
            hd2 = DCH // 2
            wq_r = wq.rearrange("(dc p) f -> p dc f", p=P)
            for h in range(2):
                nc.sync.dma_start(out=wq_sb[:, h * hd2:(h + 1) * hd2, :],
                                  in_=wq_r[:, h * hd2:(h + 1) * hd2, :])
            nc.scalar.dma_start(out=bq_sb,
                                in_=bq.rearrange("(o p) -> p o", p=P))
            nc.scalar.dma_start(out=wk_sb,
                                in_=wk.rearrange("(dc p) f -> p dc f", p=P))
            nc.scalar.dma_start(out=bk_sb,
                                in_=bk.rearrange("(o p) -> p o", p=P))
            # wp is first needed by the output-projection units, which are
            # all deferred into the last chunk's attention window -- load last
            wp_sb = big.tile([P, NLC, NDOUT, DOUT_CH], pj_dt, tag="wp_sb")

            for _rep in range(reps):
                qT = big.tile([P, NFB, T], qk_dt, tag="qT")     # [f%128, fb, tok]
                kT = big.tile([P, NFB, T], qk_dt, tag="kT")
                # v_aug[:, j, h] = [v (64 cols) | ones (64 cols)]: the AV
                # matmul replicates the softmax denominator on PSUM
                # partitions 64..127, feeding a plain [64, TCH] divide
                v_aug = big.tile([P, NTB, HLOC, 2 * HD], av_dt, tag="v")
                yTs = [big.tile([P, T], pj_dt, tag=f"yT{lc}", name=f"yT{lc}")
                       for lc in range(NLC)]

                nc.gpsimd.memset(v_aug[:, :, :, HD:2 * HD], 1.0)

                # ---- filler machinery: generators yielding one PE matmul
                # (plus trailing evacuation ops) per step.  `fillers` holds
                # next-chunk projection units (deadline: start of next
                # chunk's attention); `dfill` holds output-projection units,
                # deferred into the last chunk's attention window, which
                # would otherwise run out of PE filler work ----
                fillers = []
                dfill = []

                class Unit:
                    __slots__ = ("gen", "done")

                    def __init__(self, gen):
                        self.gen = gen
                        self.done = False

                def _step(q):
                    while q:
                        try:
                            next(q[0].gen)
                            return True
                        except StopIteration:
                            q[0].done = True
                            q.pop(0)
                    return False

                def pump(n, allow_d=False):
                    while n > 0:
                        if _step(fillers) or (allow_d and _step(dfill)):
                            n -= 1
                        else:
                            return

                def pump_all(q=None):
                    while _step(fillers if q is None else q):
                        pass

                def gen_vproj(c, tb, xT):
                    pv = flex.tile([P, 512], F32, tag="flex")
                    for d in range(DCH):
                        nc.tensor.matmul(
                            pv[:, :FQ],
                            xT[:, d, tb * P:(tb + 1) * P],
                            wv_sb[:, d, :],
                            start=(d == 0), stop=(d == DCH - 1),
                        )
                        if d < DCH - 1:
                            yield
                    nc.vector.tensor_copy(
                        out=v_aug[:, c * TBC + tb, :, 0:HD],
                        in_=pv[:, :FQ].rearrange("p (h d) -> p h d", d=HD),
                    )
                    yield

                def gen_qkproj(w_sb, bias_sb, dstT, c, fb, xT):
                    pq = flex.tile([P, TCH], F32, tag="flex")
                    t0 = c * TCH
                    for d in range(DCH):
                        nc.tensor.matmul(
                            pq[:, :TCH],
                            w_sb[:, d, fb * P:(fb + 1) * P],
                            xT[:, d, :],
                            start=(d == 0), stop=(d == DCH - 1),
                        )
                        if d < DCH - 1:
                            yield
                    nc.vector.tensor_scalar_add(
                        out=dstT[:, fb, t0:t0 + TCH], in0=pq[:, :TCH],
                        scalar1=bias_sb[:, fb:fb + 1],
                    )
                    yield

                def gen_dunit(tbg, o, on_act=False):
                    po = flex.tile([P, 512], F32, tag="flex")
                    for lc in range(NLC):
                        nc.tensor.matmul(
                            po[:, :DOUT_CH],
                            yTs[lc][:, tbg * P:(tbg + 1) * P],
                            wp_sb[:, lc, o, :],
                            start=(lc == 0), stop=(lc == NLC - 1),
                        )
                        if lc < NLC - 1:
                            yield
                    ot = outp.tile([P, DOUT_CH], out_dt, tag="out")
                    if on_act:
                        # ACT is idle once the last exp retires; splitting the
                        # tail evacuations (and their stores) between ACT and
                        # DVE/SP shortens the final drain
                        nc.scalar.activation(
                            out=ot, in_=po[:, :DOUT_CH],
                            func=mybir.ActivationFunctionType.Copy)
                        dma_eng = nc.scalar
                    else:
                        nc.vector.tensor_copy(out=ot, in_=po[:, :DOUT_CH])
                        dma_eng = nc.sync
                    dma_eng.dma_start(
                        out=out[tbg * P:(tbg + 1) * P,
                                o * DOUT_CH:(o + 1) * DOUT_CH],
                        in_=ot,
                    )
                    yield

                def stage_b_direct(c, xtiles):
                    """chunk-0 projections, emitted back-to-back.  The v
                    projection runs d-major across 4 parallel PSUM groups so
                    each arriving wv slice is consumed immediately."""
                    pvs = [flex.tile([P, 512], F32, tag="flex",
                                     name=f"pv{tb}")
                           for tb in range(TBC)]
                    for d in range(DCH):
                        for tb in range(TBC):
                            nc.tensor.matmul(
                                pvs[tb][:, :FQ],
                                xtiles[:, d, tb * P:(tb + 1) * P],
                                wv_sb[:, d, :],
                                start=(d == 0), stop=(d == DCH - 1),
                            )
                    for tb in range(TBC):
                        nc.vector.tensor_copy(
                            out=v_aug[:, c * TBC + tb, :, 0:HD],
                            in_=pvs[tb][:, :FQ].rearrange(
                                "p (h d) -> p h d", d=HD),
                        )
                    prereq = {}
                    for fb in range(NFB):
                        units = [
                            Unit(gen_qkproj(wq_sb, bq_sb, qT, c, fb, xtiles)),
                            Unit(gen_qkproj(wk_sb, bk_sb, kT, c, fb, xtiles)),
                        ]
                        for u in units:
                            for _ in u.gen:
                                pass
                            u.done = True
                        prereq[fb] = units
                    return prereq

                def queue_bnext(c):
                    """queue chunk c's x load + projections as fillers."""
                    t0 = c * TCH
                    xT = xtp.tile([P, DCH, TCH], x_dt, tag="xT")
                    h = TCH // 2
                    nc.sync.dma_start(out=xT[:, :, 0:h],
                                      in_=xbT[:, :, t0:t0 + h])
                    nc.sync.dma_start(out=xT[:, :, h:TCH],
                                      in_=xbT[:, :, t0 + h:t0 + TCH])
                    for tb in range(TBC):
                        fillers.append(Unit(gen_vproj(c, tb, xT)))
                    for (w_sb, bias_sb, dstT) in (
                            (wq_sb, bq_sb, qT), (wk_sb, bk_sb, kT)):
                        for fb in range(NFB):
                            fillers.append(Unit(
                                gen_qkproj(w_sb, bias_sb, dstT, c, fb, xT)))

                def stage_c_pair(c, hp, allow_d=False):
                    """causal attention for query chunk c, head pair hp.

                    Software-pipelined: QK(j+1) is emitted before AV(j) so the
                    in-order PE queue computes the next block's scores while
                    ACT runs exp on the current block; fillers are pumped
                    between QK(j+1) and AV(j) to cover exp latency."""
                    q0 = c * TCH
                    nj = TBC * c + TBC

                    def q_off_of(j):
                        r = j - TBC * c
                        return 0 if (r < 0 or not trim) else P * r

                    # head B's score region starts at a PSUM bank boundary:
                    # the paired QK matmuls execute concurrently (row tiles)
                    # and must not write the same 2KB bank
                    SCB = max(TCH, 512)

                    def emit_qk(j):
                        q_off = q_off_of(j)
                        sc = ps.tile([P, 2 * SCB], F32, tag="sc")
                        nc.tensor.matmul(
                            sc[:, q_off:TCH],
                            kT[0:HD, hp, j * P:(j + 1) * P],
                            qT[0:HD, hp, q0 + q_off:q0 + TCH],
                            start=True, stop=True,
                        )
                        # timing experiment: qk_nopair puts head B's QK on
                        # the same PE row tile as head A's (serializing the
                        # pair; head-B numerics wrong)
                        kB = kT[0:HD] if qk_nopair else kT[HD:P]
                        qB = qT[0:HD] if qk_nopair else qT[HD:P]
                        nc.tensor.matmul(
                            sc[:, SCB + q_off:SCB + TCH],
                            kB[:, hp, j * P:(j + 1) * P],
                            qB[:, hp, q0 + q_off:q0 + TCH],
                            start=True, stop=True,
                        )
                        e = est.tile([P, 2 * TCH], av_dt, tag="est")
                        nh = 1 if exp_half else 2   # timing experiment only
                        sc_v = sc.rearrange(
                            "p (t q) -> p t q", q=SCB)[:, 0:nh, q_off:TCH]
                        e_v = e.rearrange(
                            "p (t q) -> p t q", t=2)[:, 0:nh, q_off:TCH]
                        nc.scalar.activation(
                            out=e_v, in_=sc_v,
                            func=mybir.ActivationFunctionType.Exp,
                            scale=scale,
                        )
                        r = j - TBC * c
                        if r >= 0:
                            w = TCH - q_off
                            e_2v = e.rearrange(
                                "p (t q) -> p t q", t=2)[:, :, q_off:TCH]
                            m_2v = masks[r][:, q_off:TCH].rearrange(
                                "p (o w) -> p o w", o=1).broadcast_to(
                                [P, 2, w])
                            nc.vector.tensor_mul(out=e_2v, in0=e_2v, in1=m_2v)
                        return e

                    py_a = flex.tile([P, TCH], F32, tag="flex")
                    py_b = flex.tile([P, TCH], F32, tag="flex")
                    lead = qk_lead
                    e_pipe = [emit_qk(jj) for jj in range(min(lead, nj))]
                    for j in range(nj):
                        e_cur = e_pipe.pop(0)
                        if j + lead < nj:
                            e_pipe.append(emit_qk(j + lead))
                        # deficit-matched fill: the last chunk draws only on
                        # the finite deferred output-projection queue, so
                        # pace it to the actual ACT-PE gap (~1 matmul/step)
                        pump(1 if allow_d else fill_per_j, allow_d=allow_d)
                        q_off = q_off_of(j)
                        nc.tensor.matmul(
                            py_a[:, q_off:TCH],
                            v_aug[:, j, 2 * hp, :],
                            e_cur[:, q_off:TCH],
                            start=(j == 0), stop=(j == nj - 1),
                        )
                        nc.tensor.matmul(
                            py_b[:, q_off:TCH],
                            v_aug[:, j, 2 * hp + 1, :],
                            e_cur[:, TCH + q_off:2 * TCH],
                            start=(j == 0), stop=(j == nj - 1),
                        )
                    # normalize: DVE may read only ONE input from PSUM, so
                    # reciprocal the replicated denominator rows into SBUF
                    # scratch, then multiply against the PSUM accumulator
                    fb = hp
                    for (py_h, p0, p1) in ((py_a, 0, HD), (py_b, HD, P)):
                        rc = small.tile([HD, TCH], F32, tag="rc")
                        nc.vector.reciprocal(out=rc, in_=py_h[HD:P, :])
                        nc.vector.tensor_mul(
                            out=yTs[fb][p0:p1, q0:q0 + TCH],
                            in0=py_h[0:HD, :], in1=rc)

                for c in range(NTC):
                    last = c == NTC - 1
                    prereq0 = None
                    if c == 0:
                        if _rep == 0:
                            prereq0 = stage_b_direct(0, xpre)
                        else:
                            xT0 = xtp.tile([P, DCH, TCH], x_dt, tag="xT")
                            nc.sync.dma_start(out=xT0, in_=xbT[:, :, 0:TCH])
                            prereq0 = stage_b_direct(0, xT0)
                    if c + 1 < NTC:
                        queue_bnext(c + 1)
                    if c == 0 and _rep == 0:
                        nc.scalar.dma_start(
                            out=wp_sb,
                            in_=wp.rearrange("(lc p) (o q) -> p lc o q",
                                             p=P, q=DOUT_CH))
                    pump(2, allow_d=last)   # cover the chunk-start exp bubble
                    for hp in range(NPAIR):
                        if prereq0 is not None:
                            # pair hp reads qT/kT feature block hp: those
                            # writes MUST be emitted before the QK that reads
                            # them (emission order defines the dependency
                            # direction)
                            while any(not u.done for u in prereq0[hp]):
                                if not _step(fillers):
                                    break
                        stage_c_pair(c, hp, allow_d=last)
                        pump(pair_fill, allow_d=last)
                    # chunk c's projections for c+1 must be fully emitted
                    # before chunk c+1's attention starts
                    pump_all()
                    for tb in range(TBC):
                        for o in range(NDOUT):
                            dfill.append(Unit(gen_dunit(
                                c * TBC + tb, o,
                                on_act=last and (tb + o) % 2 == 0)))
                pump_all(dfill)

    nc.finalize()
    return nc


DEFAULT_CFG = dict(est_bufs=4)
BUILD = build_nc3

_NC_CACHE = {}


def _get_nc():
    if "nc" not in _NC_CACHE:
        _NC_CACHE["nc"] = build_nc3()
    return _NC_CACHE["nc"]


def _core_inputs(inputs, x_bf16=True, pj_bf16=True):
    import ml_dtypes
    bf = ml_dtypes.bfloat16
    xdt = bf if x_bf16 else np.float32
    pdt = bf if pj_bf16 else np.float32
    x = np.ascontiguousarray(np.asarray(inputs["x"], dtype=np.float32))
    W = np.asarray(inputs["W_attn"], dtype=np.float32)
    ba = np.asarray(inputs["b_attn"], dtype=np.float32)
    Wp = np.asarray(inputs["W_proj"], dtype=np.float32)
    maps = []
    for c in range(N_CORES):
        b, g = c // N_GROUPS, c % N_GROUPS
        s = slice(g * FQ, (g + 1) * FQ)
        # xbT[p, dc, t] = x[b][t, dc*128 + p] (see build_nc3: host-side
        # transpose replaces the XBAR transpose DMAs)
        xT = x[b].reshape(T, D // 128, 128).transpose(2, 1, 0)
        maps.append({
            "xbT": np.ascontiguousarray(xT).astype(xdt),
            "wq": np.ascontiguousarray(W[:, 0:D][:, s]).astype(xdt),
            "wk": np.ascontiguousarray(W[:, D:2 * D][:, s]).astype(xdt),
            "wv": np.ascontiguousarray(W[:, 2 * D:3 * D][:, s]).astype(xdt),
            "bq": np.ascontiguousarray(ba[0:D][s]),
            "bk": np.ascontiguousarray(ba[D:2 * D][s]),
            "wp": np.ascontiguousarray(Wp[s, :]).astype(pdt),
        })
    return maps


def kernel(**inputs) -> np.ndarray:
    global LAST_RESULTS
    nc = _get_nc()
    maps = _core_inputs(inputs)
    res = run_bass_kernel_spmd(
        nc, maps, list(range(N_CORES)), trace=TRACE,
        trace_cores=list(range(N_CORES)) if TRACE else None,
    )
    LAST_RESULTS = res
    bp = np.asarray(inputs["b_proj"], dtype=np.float32)
    # v-bias contribution, exact in f32: bv @ W_proj (see kernel docstring)
    bv = np.asarray(inputs["b_attn"], dtype=np.float32)[2 * D:3 * D]
    bvp = bv @ np.asarray(inputs["W_proj"], dtype=np.float32)
    out = np.empty((B, T, D), dtype=np.float32)
    for b in range(B):
        acc = res.results[b * N_GROUPS]["out"].astype(np.float32).copy()
        for g in range(1, N_GROUPS):
            acc += res.results[b * N_GROUPS + g]["out"]
        out[b] = acc + bp + bvp
    return out

